# revision 7
# baseline (speedup 1.0000x reference)
"""Cross-attention kernel for TRN2 (8 NeuronCores, data-parallel over batch).

Problem (per batch element b):
    s[e,t] = sum_d enc[b,e,d] * dec[b,t,d]
    a      = softmax(s, axis=e)
    out[b,t,d] = sum_e a[e,t] * enc[b,e,d]

Per-core layout (B=8 -> one batch element per core), Plan C:
  - mm1 computes s in [t_block=128, e] layout with f32r inputs: lhsT = decT
    (d-major), rhs = encT (d-major); contraction over d on the PE partition
    axis. n-outer loop so each 512-wide psum bank finishes early and the
    per-bank max reduction overlaps the remaining mm1 matmuls.
  - softmax along the free axis: per-bank DVE reduce_max -> fold ->
    single 2048-wide ACT exp with per-partition bias, output in bf16,
    row sum Z from the ACT accumulator.
  - p is transposed via ONE xbar DMA-transpose per t-block (bf16 only;
    SP queue) -> no PE transposes, no PSUM-evacuation copies.
  - mm2 in bf16: lhsT = pT tiles, rhs = enc in natural [e,d] layout
    (host-cast bf16 input). 1/Z applied during PSUM evacuation.

Host side transposes enc/dec once and casts enc to bf16 (numpy) so the
device never transposes or casts inputs.
"""

import numpy as np

import concourse.bass as bass
import concourse.tile as tile
from concourse import masks, mybir
from concourse.bass_utils import run_bass_kernel_spmd

F32 = mybir.dt.float32
F32R = mybir.dt.float32r
BF16 = mybir.dt.bfloat16


def _fast_drain_and_barrier(self, tick_clock, wait_clock):
    # Tile tail without the second all-engine barrier: NEFF completion
    # already waits for every engine queue to drain, and the gpsimd sem/dma
    # clears are ordered within the gpsimd queue, so re-execution still sees
    # cleared semaphores. Saves a few us of fixed tail per execution.
    from concourse.vector_clock import ScopedClock
    nc = self.nc
    drain_inst = nc.sync.drain()
    wait_clock.add_sem_waits(drain_inst.ins,
                             ScopedClock({None: tick_clock.global_clock}))
    nc.all_engine_barrier()
    popped = nc._tile_sem_poison_stack.pop()
    assert popped is self._sem_poison
    nc.clear_and_free_semaphores(list(self.sems.allocated().values()))


tile.TileContext._drain_and_barrier = _fast_drain_and_barrier

B, S_ENC, S_DEC, D = 8, 2048, 2048, 512
N_CORES = 8

MM1_DT = F32R  # scores need ~1e-4; bf16 would be too coarse through exp


def _split_multi_waits(nc):
    """This walrus build rejects any instruction with >1 sync wait. Hoist
    surplus waits onto single-wait same-engine NOPs placed just before."""
    for f in nc.m.functions:
        for bb in f.blocks:
            new_list = []
            changed = False
            for inst in bb.instructions:
                si = inst.sync_info
                waits = list(si.on_wait) if si and si.on_wait else []
                if len(waits) > 1:
                    changed = True
                    for w in waits[:-1]:
                        nop = mybir.InstNoOp(
                            name=nc.get_next_instruction_name(),
                            engine=inst.engine,
                            sync_info=mybir.SyncInfo(on_wait=[w], on_update=[]),
                            bass_nofuse=True,
                        )
                        nc.register_instruction(nop, overwrite=True)
                        new_list.append(nop)
                    si.on_wait = waits[-1:]
                new_list.append(inst)
            if changed:
                bb.instructions = new_list


def attention_body_c(tc, out, encT, decT, encb, E, T, Dd, mm1_dt, dbg=None):
    nc = tc.nc
    KD = Dd // 128   # d-tiles (contraction of mm1)
    NE = E // 512    # e-chunks of mm1 output (psum bank-sized)
    JT = E // 128    # e-tiles (contraction of mm2)
    TB = T // 128    # t row-blocks
    Exp = mybir.ActivationFunctionType.Exp
    X = mybir.AxisListType.X

    with (
        tc.tile_pool(name="resident", bufs=1) as res_pool,
        tc.tile_pool(name="work", bufs=2) as work,
        tc.tile_pool(name="ps_s", bufs=1, space="PSUM") as ps_s,
        tc.tile_pool(name="ps_t", bufs=2, space="PSUM") as ps_t,
        tc.tile_pool(name="ps_c", bufs=2, space="PSUM") as ps_c,
    ):
        encTt = res_pool.tile([128, KD, E], mm1_dt)
        decTt = res_pool.tile([128, KD, T], mm1_dt)
        encS = res_pool.tile([128, JT, Dd], BF16)
        ident = res_pool.tile([128, 128], BF16)

        # Startup loads: priority-ordered, fine-grained, split across the
        # three DMA-capable queues so mm1(tb=0, bank 0) — which needs only
        # decT[:, :, 0:128] + encT e-chunk 0 (1.25 MB) — can fire ASAP.
        # exp runs on the scalar queue, so scalar gets only early loads.
        def enc_k_e(k, n):   # encTt[:, k, e-chunk n] <- encT rows, e-cols
            return (encTt[:, k, n * 512:(n + 1) * 512],
                    encT[k * 128:(k + 1) * 128, n * 512:(n + 1) * 512])

        enc_r = encb.rearrange("(g p) d -> p g d", p=128)
        GJ = JT // 4
        # sync: decT t0-block, encT e0(k0,k1), decT c0 rest, encS g0/g1, decT c1
        for k in range(KD):
            nc.sync.dma_start(decTt[:, k, 0:128], decT[k * 128:(k + 1) * 128, 0:128])
        nc.sync.dma_start(*enc_k_e(0, 0))
        nc.sync.dma_start(*enc_k_e(1, 0))
        for k in range(KD):
            nc.sync.dma_start(decTt[:, k, 128:512], decT[k * 128:(k + 1) * 128, 128:512])
        for g in (0, 1):
            nc.sync.dma_start(encS[:, g * GJ:(g + 1) * GJ, :],
                              enc_r[:, g * GJ:(g + 1) * GJ, :])
        for k in range(KD):
            nc.sync.dma_start(decTt[:, k, 512:1024], decT[k * 128:(k + 1) * 128, 512:1024])
        # scalar: encT e0(k2,k3), e1, e2 (done early; queue then free for exp)
        nc.scalar.dma_start(*enc_k_e(2, 0))
        nc.scalar.dma_start(*enc_k_e(3, 0))
        for k in range(KD):
            nc.scalar.dma_start(*enc_k_e(k, 1))
        for k in range(KD):
            nc.scalar.dma_start(*enc_k_e(k, 2))
        # gpsimd: encT e3, encS g2/g3, decT c2, c3
        for k in range(KD):
            nc.gpsimd.dma_start(*enc_k_e(k, 3))
        for g in (2, 3):
            nc.gpsimd.dma_start(encS[:, g * GJ:(g + 1) * GJ, :],
                                enc_r[:, g * GJ:(g + 1) * GJ, :])
        for c_ in (2, 3):
            for k in range(KD):
                nc.gpsimd.dma_start(decTt[:, k, c_ * 512:(c_ + 1) * 512],
                                    decT[k * 128:(k + 1) * 128, c_ * 512:(c_ + 1) * 512])
        identf = res_pool.tile([128, 128], F32)
        masks.make_identity(nc, identf[:])
        nc.vector.tensor_copy(ident[:], identf[:])

        state = None
        for tb in range(TB + 1):
            cur = None
            if tb < TB:
                # mm1: s[t_block, e]; n-outer so psum bank n completes after
                # its KD matmuls -> per-bank max overlaps remaining mm1.
                psum_s = ps_s.tile([128, E], F32, tag="s")
                pmax = work.tile([128, NE], F32, tag="pmax")
                for n in range(NE):
                    for k in range(KD):
                        nc.tensor.matmul(
                            psum_s[:, n * 512:(n + 1) * 512],
                            decTt[:, k, tb * 128:(tb + 1) * 128],
                            encTt[:, k, n * 512:(n + 1) * 512],
                            start=(k == 0),
                            stop=(k == KD - 1),
                        )
                    nc.vector.reduce_max(out=pmax[:, n:n + 1],
                                         in_=psum_s[:, n * 512:(n + 1) * 512],
                                         axis=X)
                negm = work.tile([128, 1], F32, tag="negm")
                nc.vector.reduce_max(out=negm[:], in_=pmax[:], axis=X, negate=True)
                # exp in bank-sized chunks: psum bank n is released right
                # after its chunk, so mm1(tb+1) starts ~1.5us earlier than
                # with a single 2048-wide exp.
                p = work.tile([128, E], BF16, tag="p")
                zp = work.tile([128, NE], F32, tag="zp")
                for n in range(NE):
                    nc.scalar.activation(out=p[:, n * 512:(n + 1) * 512],
                                         in_=psum_s[:, n * 512:(n + 1) * 512],
                                         func=Exp, bias=negm[:], scale=1.0,
                                         accum_out=zp[:, n:n + 1])
                z = work.tile([128, 1], F32, tag="z")
                nc.vector.reduce_sum(out=z[:], in_=zp[:], axis=X)
                rz = work.tile([128, 1], F32, tag="rz")
                nc.vector.reciprocal(rz[:], z[:])
                cur = (p, rz, tb)

            if state is not None:
                pp, rzp, tbp = state
                # PE transposes in bf16, 8 per PSUM bank; each bank drains
                # with ONE wide copy instead of 16 small ones.
                pT = work.tile([128, JT, 128], BF16, tag="pT")
                for h in range(2):
                    pst = ps_t.tile([128, 8, 128], BF16, tag="pt")
                    for jj in range(8):
                        j = h * 8 + jj
                        nc.tensor.transpose(pst[:, jj, :],
                                            pp[:, j * 128:(j + 1) * 128],
                                            ident[:])
                    if h == 0:
                        nc.scalar.copy(pT[:, 0:8, :], pst[:])
                    else:
                        nc.vector.tensor_copy(pT[:, 8:16, :], pst[:])
                psum_c = ps_c.tile([128, Dd], F32, tag="c")
                for j in range(JT):
                    nc.tensor.matmul(psum_c[:], pT[:, j, :], encS[:, j, :],
                                     start=(j == 0), stop=(j == JT - 1))
                c = work.tile([128, Dd], F32, tag="c_sb")
                nc.vector.tensor_scalar_mul(c[:], psum_c[:], rzp[:])
                nc.gpsimd.dma_start(out[tbp * 128:(tbp + 1) * 128, :], c[:])

            state = cur


def build(E=S_ENC, T=S_DEC, Dd=D, mm1_dt=MM1_DT):
    nc = bass.Bass("TRN2", target_bir_lowering=False, debug=False)
    encT = nc.dram_tensor("encT", [Dd, E], mm1_dt, kind="ExternalInput").ap()
    decT = nc.dram_tensor("decT", [Dd, T], mm1_dt, kind="ExternalInput").ap()
    encb = nc.dram_tensor("encb", [E, Dd], BF16, kind="ExternalInput").ap()
    out = nc.dram_tensor("out", [T, Dd], F32, kind="ExternalOutput").ap()
    with tile.TileContext(nc) as tc:
        attention_body_c(tc, out, encT, decT, encb, E, T, Dd, mm1_dt)
    _split_multi_waits(nc)
    return nc


def make_in_maps(enc_output, dec_output):
    enc_output = np.asarray(enc_output, dtype=np.float32)
    dec_output = np.asarray(dec_output, dtype=np.float32)
    bf16 = mybir.dt.np(BF16)
    in_maps = []
    for b in range(B):
        in_maps.append({
            "encT": np.ascontiguousarray(enc_output[b].T),
            "decT": np.ascontiguousarray(dec_output[b].T),
            "encb": np.ascontiguousarray(enc_output[b]).astype(bf16),
        })
    return in_maps


_nc_cache = {}


def _get_nc():
    key = (MM1_DT,)
    if key not in _nc_cache:
        _nc_cache[key] = build()
    return _nc_cache[key]


def kernel(enc_output, dec_output):
    nc = _get_nc()
    in_maps = make_in_maps(enc_output, dec_output)
    last_err = None
    for _attempt in range(3):
        try:
            res = run_bass_kernel_spmd(nc, in_maps, list(range(N_CORES)))
            return np.stack([res.results[b]["out"] for b in range(B)])
        except Exception as e:  # transient device wedge -> retry
            last_err = e
    raise last_err


# revision 33
# speedup vs baseline: 1.6917x; 1.6917x over previous
"""Cross-attention kernel for TRN2 (8 NeuronCores, data-parallel over batch).

Problem (per batch element b):
    s[e,t] = sum_d enc[b,e,d] * dec[b,t,d]
    a      = softmax(s, axis=e)
    out[b,t,d] = sum_e a[e,t] * enc[b,e,d]

Per-core layout (B=8 -> one batch element per core):
  - mm1 computes s in [t_block=128, e] layout with fp16 inputs; contraction
    over d on the PE partition axis. psum_s is split into two 2-bank halves
    (bufs=2) so mm1(tb+1)'s first half starts as soon as exp-half-0 of tb
    has drained its banks; per-bank DVE maxes overlap the matmuls.
  - softmax along the free axis: fold of per-bank maxes -> two 1024-wide
    ACT exp chunks (bias = -max) writing fp16 p, Z via the ACT accumulator;
    z/rz emitted after the mm2 block (in-order engine queues: order by
    data readiness to avoid head-of-line blocking).
  - p transposed on the PE (fp16, 1 cyc/row), 8 blocks packed per PSUM
    bank, each bank drained by ONE wide copy (DVE + ACT).
  - mm2 in fp16: lhsT = pT tiles, rhs = enc in natural [e,d] layout
    (host-cast fp16). 1/Z applied during PSUM evacuation.
  - Startup DMAs are consolidated (one DMA per multi-k slab) and priority
    ordered across the sync/scalar/gpsimd queues in consumption order.

Host side transposes enc/dec once and casts to fp16 (numpy) so the device
never transposes or casts inputs.

NOTE: do NOT use the xbar DMA-transpose (dma_start(transpose=True)) here:
in this kernel's full instruction mix it deterministically rounds bf16/fp16
values on even output partitions to their top 4 bits (f32r-style low-half
truncation of 32-bit pairs), though it is exact in every isolation test.
"""

import numpy as np

import concourse.bass as bass
import concourse.tile as tile
from concourse import masks, mybir
from concourse.bass_utils import run_bass_kernel_spmd

F32 = mybir.dt.float32
F32R = mybir.dt.float32r
# 16-bit dtype for p / pT / enc(mm2 rhs): fp16 (10-bit mantissa) is more
# accurate than bf16 here and equally fast on the PE.
BF16 = mybir.dt.float16


def _fast_drain_and_barrier(self, tick_clock, wait_clock):
    # Tile tail without the second all-engine barrier: NEFF completion
    # already waits for every engine queue to drain, and the gpsimd sem/dma
    # clears are ordered within the gpsimd queue, so re-execution still sees
    # cleared semaphores. Saves a few us of fixed tail per execution.
    from concourse.vector_clock import ScopedClock
    nc = self.nc
    drain_inst = nc.sync.drain()
    wait_clock.add_sem_waits(drain_inst.ins,
                             ScopedClock({None: tick_clock.global_clock}))
    nc.all_engine_barrier()
    popped = nc._tile_sem_poison_stack.pop()
    assert popped is self._sem_poison
    nc.clear_and_free_semaphores(list(self.sems.allocated().values()))


tile.TileContext._drain_and_barrier = _fast_drain_and_barrier

B, S_ENC, S_DEC, D = 8, 2048, 2048, 512
N_CORES = 8

MM1_DT = mybir.dt.float16  # fp16 scores: ~0.02 abs err through exp; 2x less DMA


def _split_multi_waits(nc):
    """This walrus build rejects any instruction with >1 sync wait. Hoist
    surplus waits onto single-wait same-engine NOPs placed just before."""
    for f in nc.m.functions:
        for bb in f.blocks:
            new_list = []
            changed = False
            for inst in bb.instructions:
                si = inst.sync_info
                waits = list(si.on_wait) if si and si.on_wait else []
                if len(waits) > 1:
                    changed = True
                    for w in waits[:-1]:
                        nop = mybir.InstNoOp(
                            name=nc.get_next_instruction_name(),
                            engine=inst.engine,
                            sync_info=mybir.SyncInfo(on_wait=[w], on_update=[]),
                            bass_nofuse=True,
                        )
                        nc.register_instruction(nop, overwrite=True)
                        new_list.append(nop)
                    si.on_wait = waits[-1:]
                new_list.append(inst)
            if changed:
                bb.instructions = new_list


def attention_body_c(tc, out, encT, decT, encb, E, T, Dd, mm1_dt, dbg=None):
    nc = tc.nc
    KD = Dd // 128   # d-tiles (contraction of mm1)
    NE = E // 512    # e-chunks of mm1 output (psum bank-sized)
    JT = E // 128    # e-tiles (contraction of mm2)
    TB = T // 128    # t row-blocks
    Exp = mybir.ActivationFunctionType.Exp
    X = mybir.AxisListType.X

    with (
        tc.tile_pool(name="resident", bufs=1) as res_pool,
        tc.tile_pool(name="work", bufs=2) as work,
        tc.tile_pool(name="ps_s", bufs=2, space="PSUM") as ps_s,
        tc.tile_pool(name="ps_t", bufs=2, space="PSUM") as ps_t,
        tc.tile_pool(name="ps_c", bufs=2, space="PSUM") as ps_c,
    ):
        encTt = res_pool.tile([128, KD, E], mm1_dt)
        decTt = res_pool.tile([128, KD, T], mm1_dt)
        encS = res_pool.tile([128, JT, Dd], BF16)
        ident = res_pool.tile([128, 128], BF16)

        # Startup loads: priority-ordered, CONSOLIDATED (each DMA covers all
        # 4 k-tiles via a rearranged source AP — DMA issue rate is the
        # startup limiter, ~0.7us per issue per queue). mm1(tb=0, bank 0)
        # needs only decT[:, :, 0:128] + encT e-chunk 0 (1.25 MB).
        # PE p-state warmup: the PE needs ~3us of continuous work to reach
        # full clock, and it would otherwise idle until the first input DMAs
        # land (~7us). Burn that window with dummy f32 matmuls (4 cyc/row)
        # accumulating into a scratch psum tile so mm1(tb=0) starts at 2.4GHz.
        wsrc = res_pool.tile([128, 128], F32)
        nc.vector.memset(wsrc[:], 1.0)
        # NOTE: longer bursts (44) REGRESS ~25us — the scheduler/psum-pool
        # interaction delays the first real block. 16 is tuned.
        NWARM = 16
        warm = ps_s.tile([128, E // 2], F32, tag="s")
        for i in range(NWARM):
            nc.tensor.matmul(warm[:, 0:128], wsrc[:], wsrc[:],
                             start=(i == 0), stop=(i == NWARM - 1))

        encR = encT.rearrange("(k p) e -> p k e", p=128)   # [128, KD, E]
        decR = decT.rearrange("(k p) t -> p k t", p=128)   # [128, KD, T]
        enc_r = encb.rearrange("(g p) d -> p g d", p=128)
        GJ = JT // 4
        # sync: decT t0, encT e0/e1/e2 (k0,k1), decT rest-of-c0, encS, c1
        nc.sync.dma_start(decTt[:, :, 0:128], decR[:, :, 0:128])
        nc.sync.dma_start(encTt[:, 0:2, 0:512], encR[:, 0:2, 0:512])
        nc.sync.dma_start(encTt[:, 0:2, 512:1024], encR[:, 0:2, 512:1024])
        nc.sync.dma_start(encTt[:, 0:2, 1024:1536], encR[:, 0:2, 1024:1536])
        nc.sync.dma_start(decTt[:, :, 128:512], decR[:, :, 128:512])
        nc.sync.dma_start(encS[:, 0:2 * GJ, :], enc_r[:, 0:2 * GJ, :])
        nc.sync.dma_start(decTt[:, :, 512:1024], decR[:, :, 512:1024])
        # scalar: encT e0/e1/e2 (k2,k3) (early; queue then free for exp)
        nc.scalar.dma_start(encTt[:, 2:4, 0:512], encR[:, 2:4, 0:512])
        nc.scalar.dma_start(encTt[:, 2:4, 512:1024], encR[:, 2:4, 512:1024])
        nc.scalar.dma_start(encTt[:, 2:4, 1024:1536], encR[:, 2:4, 1024:1536])
        # gpsimd: encT e3, encS g2/g3, decT c2, c3
        nc.gpsimd.dma_start(encTt[:, :, 1536:2048], encR[:, :, 1536:2048])
        nc.gpsimd.dma_start(encS[:, 2 * GJ:4 * GJ, :], enc_r[:, 2 * GJ:4 * GJ, :])
        nc.gpsimd.dma_start(decTt[:, :, 1024:1536], decR[:, :, 1024:1536])
        nc.gpsimd.dma_start(decTt[:, :, 1536:2048], decR[:, :, 1536:2048])
        identf = res_pool.tile([128, 128], F32)
        masks.make_identity(nc, identf[:])
        nc.vector.tensor_copy(ident[:], identf[:])

        # Pipeline: mm1+softmax(tb) overlap transposes+mm2(tb-1). psum_s is
        # split into two 2-bank halves (bufs=2) so mm1(tb+1)'s first half
        # starts as soon as exp-half-0 of tb has drained its banks.
        state = None
        for tb in range(TB + 1):
            cur = None
            if state is not None:
                # transposes + copies FIRST: their inputs (p of tb-1) are a
                # whole period old, so the PE runs them before mm1(tb) and
                # the copies drain during mm1 -> mm2 starts gap-free.
                pp, tbp = state
                pT = work.tile([128, JT, 128], BF16, tag="pT")
                for h in range(2):
                    pst = ps_t.tile([128, 8, 128], BF16, tag="pt")
                    for jj in range(8):
                        j = h * 8 + jj
                        nc.tensor.transpose(pst[:, jj, :],
                                            pp[:, j * 128:(j + 1) * 128],
                                            ident[:])
                    nc.vector.tensor_copy(pT[:, h * 8:(h + 1) * 8, :], pst[:])
            if tb < TB:
                halves = []
                pmax = work.tile([128, NE], F32, tag="pmax")
                for h in range(2):
                    psum_h = ps_s.tile([128, E // 2], F32, tag="s")
                    halves.append(psum_h)
                    for n2 in range(NE // 2):
                        n = h * (NE // 2) + n2
                        for k in range(KD):
                            nc.tensor.matmul(
                                psum_h[:, n2 * 512:(n2 + 1) * 512],
                                decTt[:, k, tb * 128:(tb + 1) * 128],
                                encTt[:, k, n * 512:(n + 1) * 512],
                                start=(k == 0),
                                stop=(k == KD - 1),
                            )
                        nc.vector.reduce_max(out=pmax[:, n:n + 1],
                                             in_=psum_h[:, n2 * 512:(n2 + 1) * 512],
                                             axis=X)
                negm = work.tile([128, 1], F32, tag="negm")
                nc.vector.reduce_max(out=negm[:], in_=pmax[:], axis=X, negate=True)
                p = work.tile([128, E], BF16, tag="p")
                zp = work.tile([128, 2], F32, tag="zp")
                for h in range(2):
                    nc.scalar.activation(out=p[:, h * 1024:(h + 1) * 1024],
                                         in_=halves[h][:], func=Exp,
                                         bias=negm[:], scale=1.0,
                                         accum_out=zp[:, h:h + 1])
                cur = (p, tb)

            if state is not None:
                psum_c = ps_c.tile([128, Dd], F32, tag="c")
                for j in range(JT):
                    nc.tensor.matmul(psum_c[:], pT[:, j, :], encS[:, j, :],
                                     start=(j == 0), stop=(j == JT - 1))
                c = work.tile([128, Dd], F32, tag="c_sb")
                nc.vector.tensor_scalar_mul(c[:], psum_c[:], rz[:])
                nc.gpsimd.dma_start(out[tbp * 128:(tbp + 1) * 128, :], c[:])

            if cur is not None:
                # z/rz for tb, emitted AFTER the state block: zp lands late
                # (end of exp), so this avoids DVE head-of-line blocking of
                # the pT copy / c evacuation above.
                z = work.tile([128, 1], F32, tag="z")
                nc.vector.reduce_sum(out=z[:], in_=zp[:], axis=X)
                rz = work.tile([128, 1], F32, tag="rz")
                nc.vector.reciprocal(rz[:], z[:])

            state = cur


def build(E=S_ENC, T=S_DEC, Dd=D, mm1_dt=MM1_DT):
    nc = bass.Bass("TRN2", target_bir_lowering=False, debug=False)
    encT = nc.dram_tensor("encT", [Dd, E], mm1_dt, kind="ExternalInput").ap()
    decT = nc.dram_tensor("decT", [Dd, T], mm1_dt, kind="ExternalInput").ap()
    encb = nc.dram_tensor("encb", [E, Dd], BF16, kind="ExternalInput").ap()
    out = nc.dram_tensor("out", [T, Dd], F32, kind="ExternalOutput").ap()
    with tile.TileContext(nc) as tc:
        attention_body_c(tc, out, encT, decT, encb, E, T, Dd, mm1_dt)
    _split_multi_waits(nc)
    return nc


def make_in_maps(enc_output, dec_output):
    enc_output = np.asarray(enc_output, dtype=np.float32)
    dec_output = np.asarray(dec_output, dtype=np.float32)
    t16 = mybir.dt.np(BF16)
    m16 = mybir.dt.np(MM1_DT)
    in_maps = []
    for b in range(B):
        in_maps.append({
            "encT": np.ascontiguousarray(enc_output[b].T).astype(m16),
            "decT": np.ascontiguousarray(dec_output[b].T).astype(m16),
            "encb": np.ascontiguousarray(enc_output[b]).astype(t16),
        })
    return in_maps


_nc_cache = {}


def _get_nc():
    key = (MM1_DT,)
    if key not in _nc_cache:
        _nc_cache[key] = build()
    return _nc_cache[key]


def kernel(enc_output, dec_output):
    nc = _get_nc()
    in_maps = make_in_maps(enc_output, dec_output)
    last_err = None
    for _attempt in range(3):
        try:
            res = run_bass_kernel_spmd(nc, in_maps, list(range(N_CORES)))
            return np.stack([res.results[b]["out"] for b in range(B)])
        except Exception as e:  # transient device wedge -> retry
            last_err = e
    raise last_err
